# revision 1
# baseline (speedup 1.0000x reference)
"""Trainium2 Bass kernel for BinarySplitDecoder (binary-tree leaf probabilities).

Contract: kernel(x) takes the FULL input x [65536, 1023] fp32 and returns the
FULL output [65536, 1024] fp32 (leaf probabilities of a depth-10 binary split
tree, level-major node ordering).

Sharding: pure data parallel — batch dim split evenly across 8 NeuronCores.

Per-core kernel (rows_per_core = 8192, memory-bound at ~67 MB of HBM I/O):
  - Rows processed in chunks of g*128; partition p / free-group i holds batch
    row off + p*g + i, so every chunk DMA is one contiguous 2D block. Small
    leading chunks (g=1,1,2) shorten the pipeline ramp.
  - ScalarE computes oma = 1 - x per chunk (one ACT op, off the DVE; the
    first two chunks compute it on DVE so the ramp has no ACT stage).
  - DVE walks the tree level by level: left = cur * a ; right = cur * oma,
    written interleaved (stride 2) into the next level's tile. fp32
    tensor_tensor runs in 1x DVE mode regardless of stride, so the
    interleaved store is free.
  - Loads issue from the ACT sequencer (HWDGE), stores from SP: each
    sequencer drains in order, so a store's wait (on DVE finishing chunk c)
    must not block later chunks' loads — splitting the queues decouples the
    two wait chains (measured ~22 us win).
  - GPSIMD is left idle on purpose: concurrent Pool tensor ops slow DVE ops
    ~30% (SBUF port contention, measured).
  - The computation matches the reference's fp32 operation sequence exactly
    (bitwise-identical output, no cancellation on small leaves).
"""

import numpy as np

import concourse.bacc as bacc
import concourse.bass as bass
import concourse.mybir as mybir
from concourse.tile import TileContext
from concourse.bass_utils import run_bass_kernel_spmd

TREE_DEPTH = 10
N_NODES = (1 << TREE_DEPTH) - 1  # 1023
N_LEAVES = 1 << TREE_DEPTH  # 1024
N_CORES = 8
P = 128  # SBUF partitions


def build_nc(rows_per_core: int, G: int = 4, oma_on_act: bool = True) -> bass.Bass:
    """Build the per-core Bass program.

    The kernel reads DRAM input "x" [rows_per_core, 1023] and writes
    "y" [rows_per_core, 1024].
    """
    chunk_rows = G * P
    if rows_per_core >= 4 * P + chunk_rows and (rows_per_core - 4 * P) % chunk_rows == 0:
        chunks = [1, 1, 2] + [G] * ((rows_per_core - 4 * P) // chunk_rows)
    else:
        assert rows_per_core % chunk_rows == 0
        chunks = [G] * (rows_per_core // chunk_rows)
    assert sum(g * P for g in chunks) == rows_per_core
    f32 = mybir.dt.float32

    # Bacc (not raw Bass): Bacc.compile() runs generate_event_semaphores,
    # which splits multi-wait sync onto EventSemaphore instructions (TRN2
    # instructions have a single sync-wait slot).
    nc = bacc.Bacc("TRN2", target_bir_lowering=False, debug=False)
    x = nc.declare_dram_parameter("x", [rows_per_core, N_NODES], f32, isOutput=False)
    y = nc.declare_dram_parameter("y", [rows_per_core, N_LEAVES], f32, isOutput=True)

    def x_view(off, g):
        return x[off : off + g * P, :].rearrange("(p g) n -> p (g n)", g=g, p=P)

    def y_view(off, g):
        return y[off : off + g * P, :].rearrange("(p g) m -> p (g m)", g=g, p=P)

    with TileContext(nc) as tc:
        with (
            tc.tile_pool(name="xin", bufs=3) as xp,
            tc.tile_pool(name="oma", bufs=3) as omap,
            tc.tile_pool(name="out", bufs=3) as outp,
            # bufs=2: with one buffer, chunk c+1's level-0 write must wait
            # for the level-9 reads of chunk c (WAR) — a per-chunk stall.
            tc.tile_pool(name="cur", bufs=2) as curp,
        ):
            off = 0
            for c, g in enumerate(chunks):
                if oma_on_act and c == 2:
                    # Pre-warm the ACT function table (the first ACTIVATE
                    # pays a ~2.7us table load). Emitted after the first two
                    # chunks' loads so it doesn't delay them; overlaps with
                    # their DVE work.
                    warm = curp.tile([P, 1, 2], f32, tag="cur0")
                    nc.vector.memset(warm[:], 0.0)
                    nc.scalar.activation(
                        out=warm[:],
                        in_=warm[:],
                        func=mybir.ActivationFunctionType.Copy,
                        bias=1.0,
                        scale=-1.0,
                    )

                xt = xp.tile([P, g, N_NODES], f32, tag="x")
                nc.scalar.dma_start(out=xt[:], in_=x_view(off, g))

                # oma = 1 - x for the whole chunk, one op off the DVE.
                on_act = oma_on_act and c >= 3
                oma_t = omap.tile([P, g, N_NODES], f32, tag="oma")
                if on_act:
                    nc.scalar.activation(
                        out=oma_t[:],
                        in_=xt[:],
                        func=mybir.ActivationFunctionType.Copy,
                        bias=1.0,
                        scale=-1.0,
                    )
                else:
                    nc.vector.tensor_scalar(
                        out=oma_t[:],
                        in0=xt[:],
                        scalar1=-1.0,
                        scalar2=1.0,
                        op0=mybir.AluOpType.mult,
                        op1=mybir.AluOpType.add,
                    )

                out_t = outp.tile([P, g, N_LEAVES], f32, tag="y")
                cur = None
                for d in range(TREE_DEPTH):
                    L = 1 << d
                    if d == TREE_DEPTH - 1:
                        nxt = out_t
                    else:
                        # ping-pong intermediate levels between two shared
                        # slots (sized by the largest level using each tag)
                        nxt = curp.tile([P, g, 2 * L], f32, tag=f"cur{d % 2}")
                    a = xt[:, :, L - 1 : 2 * L - 1]  # [P, g, L] level-d alphas
                    oma = oma_t[:, :, L - 1 : 2 * L - 1]
                    left = nxt[:, :, 0::2]
                    right = nxt[:, :, 1::2]
                    if d == 0:
                        # cur == 1:  left = a, right = 1 - a. On ACT chunks
                        # these tiny copies ride the scalar engine too,
                        # keeping them off the DVE critical path.
                        if on_act:
                            nc.scalar.activation(
                                out=left,
                                in_=a,
                                func=mybir.ActivationFunctionType.Copy,
                            )
                            nc.scalar.activation(
                                out=right,
                                in_=a,
                                func=mybir.ActivationFunctionType.Copy,
                                bias=1.0,
                                scale=-1.0,
                            )
                        else:
                            nc.vector.tensor_copy(out=left, in_=a)
                            nc.vector.tensor_copy(out=right, in_=oma)
                    else:
                        nc.vector.tensor_mul(out=left, in0=cur, in1=a)
                        nc.vector.tensor_mul(out=right, in0=cur, in1=oma)
                    cur = nxt

                nc.sync.dma_start(out=y_view(off, g), in_=out_t[:])
                off += g * P

    nc.compile()
    return nc


def _run(x: np.ndarray, **spmd_kwargs):
    """Shard x, run the Bass kernel on all 8 cores, return (y, BassKernelResults)."""
    x = np.ascontiguousarray(np.asarray(x, dtype=np.float32))
    B = x.shape[0]
    assert B % N_CORES == 0 and x.shape[1] == N_NODES
    rows_per_core = B // N_CORES

    nc = build_nc(rows_per_core)
    core_ids = list(range(N_CORES))
    in_maps = [
        {"x": x[i * rows_per_core : (i + 1) * rows_per_core]} for i in core_ids
    ]
    res = run_bass_kernel_spmd(nc, in_maps, core_ids, **spmd_kwargs)
    out = np.concatenate([r["y"] for r in res.results], axis=0)
    return out, res


def kernel(x: np.ndarray) -> np.ndarray:
    return _run(x)[0]



# revision 2
# speedup vs baseline: 1.8841x; 1.8841x over previous
"""Trainium2 Bass kernel for BinarySplitDecoder (binary-tree leaf probabilities).

Contract: kernel(x) takes the FULL input x [65536, 1023] fp32 and returns the
FULL output [65536, 1024] fp32 (leaf probabilities of a depth-10 binary split
tree, level-major node ordering).

Sharding: pure data parallel - batch dim split evenly across 8 NeuronCores.

Per-core kernel (rows_per_core = 8192, memory-bound):
  - fp16 I/O: the host converts x to fp16 and upcasts y back, halving HBM
    traffic to ~33.5 MB/core. Tolerance is 2e-2 relative to absmax; measured
    end-to-end error of the all-fp16 pipeline is ~1.5e-3.
  - Block (bit-reversal) layout: each level writes left-children into the
    first half and right-children into the second half of the next level's
    tile, so every DVE operand has a packed (stride-1) last dim. That keeps
    the ops out of the ~1.7x strided-write penalty AND qualifies them for the
    DVE 2x perf mode (2-byte dtype + packed). The resulting column order of
    y is bit-reversed; the host feeds alphas pre-permuted per level (so the
    device always reads them contiguously) and un-permutes y columns at the
    end. Both fixups are cheap numpy gathers on the host.
  - right = cur - left replaces right = cur * (1 - a): no "one minus x" pass,
    no oma tile, one DVE subtract instead.
  - Rows processed in chunks of g*128; partition p / free-group i holds batch
    row off + p*g + i, so every chunk DMA is one contiguous 2D block.
  - Loads issue from the ACT sequencer (HWDGE), stores from SP: each
    sequencer drains in order, so a store's wait (on DVE finishing chunk c)
    must not block later chunks' loads - splitting the queues decouples the
    two wait chains.
"""

import numpy as np

import concourse.bacc as bacc
import concourse.bass as bass
import concourse.mybir as mybir
from concourse.tile import TileContext
from concourse.bass_utils import run_bass_kernel_spmd

TREE_DEPTH = 10
N_NODES = (1 << TREE_DEPTH) - 1  # 1023
N_LEAVES = 1 << TREE_DEPTH  # 1024
N_CORES = 8
P = 128  # SBUF partitions


def _bitrev(j: int, bits: int) -> int:
    r = 0
    for _ in range(bits):
        r = (r << 1) | (j & 1)
        j >>= 1
    return r


def _input_perm() -> np.ndarray:
    """perm[k] = source column of x for device column k (level-major order,
    bit-reversed node index within each level)."""
    perm = np.empty(N_NODES, dtype=np.int64)
    for d in range(TREE_DEPTH):
        base = (1 << d) - 1
        for j in range(1 << d):
            perm[base + j] = base + _bitrev(j, d)
    return perm


def _output_perm() -> np.ndarray:
    """y[:, t] = y_dev[:, outperm[t]] (bit reversal, self-inverse)."""
    return np.array([_bitrev(t, TREE_DEPTH) for t in range(N_LEAVES)], dtype=np.int64)


_IN_PERM = _input_perm()
_OUT_PERM = _output_perm()


def build_nc(rows_per_core: int, G: int = 8, lead: tuple = (4, 4)) -> bass.Bass:
    """Build the per-core Bass program.

    Reads DRAM input "x" [rows_per_core, 1023] fp16 (columns pre-permuted
    per level) and writes "y" [rows_per_core, 1024] fp16 (columns in
    bit-reversed leaf order).
    """
    units = rows_per_core // P
    lead = tuple(g for g in lead if g < G)
    if sum(lead) <= units and (units - sum(lead)) % G == 0:
        chunks = list(lead) + [G] * ((units - sum(lead)) // G)
    else:
        assert units % G == 0
        chunks = [G] * (units // G)
    assert sum(chunks) == units
    f16 = mybir.dt.float16

    nc = bacc.Bacc("TRN2", target_bir_lowering=False, debug=False)
    x = nc.declare_dram_parameter("x", [rows_per_core, N_NODES], f16, isOutput=False)
    y = nc.declare_dram_parameter("y", [rows_per_core, N_LEAVES], f16, isOutput=True)

    def x_view(off, g):
        return x[off : off + g * P, :].rearrange("(p g) n -> p (g n)", g=g, p=P)

    def y_view(off, g):
        return y[off : off + g * P, :].rearrange("(p g) m -> p (g m)", g=g, p=P)

    with TileContext(nc) as tc:
        with (
            tc.tile_pool(name="xin", bufs=3) as xp,
            tc.tile_pool(name="out", bufs=3) as outp,
            # bufs=2: with one buffer, chunk c+1's level-0 write must wait
            # for the level-9 reads of chunk c (WAR) - a per-chunk stall.
            tc.tile_pool(name="cur", bufs=2) as curp,
        ):
            off = 0
            for g in chunks:
                xt = xp.tile([P, g, N_NODES], f16, tag="x")
                nc.scalar.dma_start(out=xt[:], in_=x_view(off, g))

                out_t = outp.tile([P, g, N_LEAVES], f16, tag="y")
                cur = None
                for d in range(TREE_DEPTH):
                    L = 1 << d
                    if d == TREE_DEPTH - 1:
                        nxt = out_t
                    else:
                        # ping-pong intermediate levels between two shared
                        # slots (sized by the largest level using each tag)
                        nxt = curp.tile([P, g, 2 * L], f16, tag=f"cur{d % 2}")
                    a = xt[:, :, L - 1 : 2 * L - 1]  # [P, g, L] level-d alphas
                    left = nxt[:, :, 0:L]
                    right = nxt[:, :, L : 2 * L]
                    if d == 0:
                        # cur == 1:  left = a, right = 1 - a.
                        nc.vector.tensor_copy(out=left, in_=a)
                        nc.vector.tensor_scalar(
                            out=right,
                            in0=a,
                            scalar1=-1.0,
                            scalar2=1.0,
                            op0=mybir.AluOpType.mult,
                            op1=mybir.AluOpType.add,
                        )
                    else:
                        nc.vector.tensor_mul(out=left, in0=cur, in1=a)
                        nc.vector.tensor_sub(out=right, in0=cur, in1=left)
                    cur = nxt

                nc.sync.dma_start(out=y_view(off, g), in_=out_t[:])
                off += g * P

    nc.compile()
    return nc


def _run(x: np.ndarray, **spmd_kwargs):
    """Shard x, run the Bass kernel on all 8 cores, return (y, BassKernelResults)."""
    x = np.asarray(x)
    B = x.shape[0]
    assert B % N_CORES == 0 and x.shape[1] == N_NODES
    rows_per_core = B // N_CORES

    # fp16 + per-level bit-reversed column order (see module docstring).
    x16 = np.ascontiguousarray(x.astype(np.float16)[:, _IN_PERM])

    nc = build_nc(rows_per_core)
    core_ids = list(range(N_CORES))
    in_maps = [
        {"x": x16[i * rows_per_core : (i + 1) * rows_per_core]} for i in core_ids
    ]
    res = run_bass_kernel_spmd(nc, in_maps, core_ids, **spmd_kwargs)
    y16 = np.concatenate([r["y"] for r in res.results], axis=0)
    out = y16[:, _OUT_PERM].astype(np.float32)
    return out, res


def kernel(x: np.ndarray) -> np.ndarray:
    return _run(x)[0]
